# revision 9
# baseline (speedup 1.0000x reference)
"""Trainium2 Bass/Tile kernel: MoE-routed per-sample dynamic 3x3 conv (stride 2).

Reference computation:
    pooled  = mean(x, HW)                     (B, Cin)
    rw      = sigmoid(pooled @ routing_w.T + routing_b)          (B, E)
    kernels = einsum('be,eoihw->boihw', rw, expert_weight)       (B,Cout,Cin,3,3)
    y[b]    = conv2d(x[b], kernels[b], stride 2, pad 1)          (B,Cout,56,56)

Sharding: data-parallel over batch across 8 NeuronCores (4 samples each);
routing/expert weights replicated.  No collectives.

Per-core plan:
  - expert_weight is loaded in natural [co, ci*9] layout and PE-transposed
    once into [ci, e, dydx, co] (lhsT layout for the conv matmuls).
  - per sample: x lives in SBUF as two half-sample "slabs" [128ci, 57, 113]
    with one zero pad row/col (stride-2 conv only ever reads the top/left
    pad).  Global-avg-pool reduces come straight off the slabs.
  - routing: pooled-col x routing_w^T matmul -> [1,E] logits; sigmoid on ACT;
    K=1 ones-matmul broadcasts the 4 gate scalars to all 128 partitions.
  - combined per-sample conv weights: W_b = sum_e rw[b,e] * E_r[e] via DVE
    fused (in0*scalar)+in1 ops.
  - conv: out[co, oh, ow] accumulated in PSUM over the 9 taps; each matmul is
    lhsT=[ci,co_tile] (float32r), rhs = strided slab view [ci, 7 rows, 56 cols]
    (N=392 >= 256 keeps float32r at full 1 row/cycle PE rate).
"""

import numpy as np

try:
    import concourse.bass as bass
except ImportError:  # toolchain not on sys.path in a fresh interpreter
    import sys

    for _p in ("/opt/trn_rl_repo", "/root/.axon_site/_ro/trn_rl_repo"):
        if _p not in sys.path:
            sys.path.insert(0, _p)
    import concourse.bass as bass

import concourse.mybir as mybir
from concourse.bacc import Bacc
from concourse.bass_utils import run_bass_kernel_spmd
from concourse.masks import make_identity
from concourse.tile import TileContext

FP32 = mybir.dt.float32
F32R = mybir.dt.float32r

N_CORES = 8
B_FULL = 32
B_SH = B_FULL // N_CORES  # 4 samples per core
CIN = 128
H = W = 112
COUT = 256
E = 4
KH = KW = 3
OH = OW = 56
HWSZ = H * W  # 12544
R = 7  # output rows per PSUM block
NBLK = 8  # blocks per (sample, co_tile)
NN = R * OW  # 392 moving dim per matmul
S_ROWS = 57  # slab rows: covers 28 output rows (2*28+1 input rows)
S_COLS = 113  # 1 left pad col + 112


_NC_CACHE = None


def build_nc():
    global _NC_CACHE
    if _NC_CACHE is not None:
        return _NC_CACHE

    # Bacc (not raw Bass): its finalize() runs the legality passes this walrus
    # build needs — move_matmul_waits_to_ldweights + generate_event_semaphores
    # (max 1 sync wait per instruction) + register allocation.
    nc = Bacc(trn_type="TRN2")
    x = nc.dram_tensor("x", [B_SH, CIN, H, W], FP32, kind="ExternalInput")
    rw_h = nc.dram_tensor("routing_w", [E, CIN], FP32, kind="ExternalInput")
    rb_h = nc.dram_tensor("routing_b", [E], FP32, kind="ExternalInput")
    ew_h = nc.dram_tensor(
        "expert_weight", [E, COUT, CIN, KH, KW], FP32, kind="ExternalInput"
    )
    y = nc.dram_tensor("y", [B_SH, COUT, OH, OW], FP32, kind="ExternalOutput")

    with TileContext(nc) as tc:
        with (
            tc.tile_pool(name="const", bufs=1) as const,
            tc.tile_pool(name="slabs", bufs=4) as slabs,
            tc.tile_pool(name="wpool", bufs=2) as wpool,
            tc.tile_pool(name="stage", bufs=2) as stage,
            tc.tile_pool(name="small", bufs=2) as small,
            tc.tile_pool(name="ps_conv", bufs=2, space="PSUM") as ps_conv,
            tc.tile_pool(name="ps_rt", bufs=2, space="PSUM") as ps_rt,
        ):
            # ---------------- one-time prep ----------------
            identity = const.tile([128, 128], FP32)
            make_identity(nc, identity)
            ones_row = const.tile([1, 128], FP32)
            nc.vector.memset(ones_row, 1.0)
            bias_row = const.tile([1, E], FP32)
            nc.sync.dma_start(out=bias_row, in_=rb_h[:].unsqueeze(0))
            rw_nat = const.tile([E, CIN], FP32)
            nc.sync.dma_start(out=rw_nat, in_=rw_h[:, :])
            rwT_ps = ps_rt.tile([128, E], FP32, tag="pr")
            nc.tensor.transpose(rwT_ps, rw_nat, identity[0:E, 0:E])
            rwT = const.tile([128, E], FP32)
            nc.scalar.copy(out=rwT, in_=rwT_ps)

            # expert weights, transposed to lhsT layout [ci, e, tap, co]
            e_r = const.tile([128, E, KH * KW, COUT], FP32)
            for e in range(E):
                for ct in range(2):
                    chunk = stage.tile(
                        [128, CIN * KH * KW], FP32, tag="st", name=f"ew_{e}_{ct}"
                    )
                    nc.sync.dma_start(
                        out=chunk,
                        in_=ew_h[e, ct * 128 : (ct + 1) * 128, :, :, :].rearrange(
                            "p a b c -> p (a b c)"
                        ),
                    )
                    ch_v = chunk.rearrange("p (ci k) -> p k ci", k=KH * KW)
                    for kk in range(KH * KW):
                        tp = ps_rt.tile(
                            [128, 128], FP32, tag="pr", name=f"tp_{e}_{ct}_{kk}"
                        )
                        nc.tensor.transpose(tp, ch_v[:, kk, :], identity)
                        dst = e_r[:, e, kk, ct * 128 : (ct + 1) * 128]
                        if kk % 2 == 0:
                            nc.vector.tensor_copy(out=dst, in_=tp)
                        else:
                            nc.scalar.copy(out=dst, in_=tp)

            # ---------------- per-sample pipeline ----------------
            for b in range(B_SH):
                slab0 = slabs.tile(
                    [128, S_ROWS, S_COLS], FP32, tag="slab", name=f"slab0_{b}"
                )
                slab1 = slabs.tile(
                    [128, S_ROWS, S_COLS], FP32, tag="slab", name=f"slab1_{b}"
                )
                # zero pads: slab0 row0 = top pad; col0 = left pad everywhere.
                # Everything that writes the slabs declares float32r output so
                # the BIR verifier accepts the f32r conv matmuls reading them.
                nc.vector.memset(slab0[:, 0:1, :], 0.0)
                nc.vector.memset(slab0[:, 1:S_ROWS, 0:1], 0.0)
                nc.vector.memset(slab1[:, :, 0:1], 0.0)
                # load x[b]: slab0 rows 1..56 <- x rows 0..55,
                #            slab1 rows 0..56 <- x rows 55..111
                for c0 in range(4):
                    r0 = 14 * c0
                    nc.sync.dma_start(
                        out=slab0[:, 1 + r0 : 15 + r0, 1:S_COLS].bitcast(F32R),
                        in_=x[b, :, r0 : r0 + 14, :].bitcast(F32R),
                    )
                for c0 in range(4):
                    r0 = 14 * c0
                    nr = 14 if c0 < 3 else 15
                    nc.sync.dma_start(
                        out=slab1[:, r0 : r0 + nr, 1:S_COLS].bitcast(F32R),
                        in_=x[b, :, 55 + r0 : 55 + r0 + nr, :].bitcast(F32R),
                    )

                # global average pool (sum; 1/HW folded into the logits op)
                pooled = small.tile([128, 3], FP32, tag="pooled", name=f"pooled_{b}")
                nc.vector.tensor_reduce(
                    out=pooled[:, 0:1],
                    in_=slab0[:, 1:S_ROWS, 1:S_COLS],
                    axis=mybir.AxisListType.XY,
                    op=mybir.AluOpType.add,
                )
                nc.vector.tensor_reduce(
                    out=pooled[:, 1:2],
                    in_=slab1[:, 1:S_ROWS, 1:S_COLS],
                    axis=mybir.AxisListType.XY,
                    op=mybir.AluOpType.add,
                )
                nc.vector.tensor_add(
                    out=pooled[:, 2:3], in0=pooled[:, 0:1], in1=pooled[:, 1:2]
                )

                # routing gates
                lg_ps = ps_rt.tile([1, E], FP32, tag="pr", name=f"lg_{b}")
                nc.tensor.matmul(lg_ps, pooled[:, 2:3], rwT, start=True, stop=True)
                lg_sb = small.tile([1, E], FP32, tag="lg", name=f"lgs_{b}")
                nc.vector.scalar_tensor_tensor(
                    out=lg_sb,
                    in0=lg_ps,
                    scalar=1.0 / HWSZ,
                    in1=bias_row,
                    op0=mybir.AluOpType.mult,
                    op1=mybir.AluOpType.add,
                )
                sig = small.tile([1, E], FP32, tag="sig", name=f"sig_{b}")
                nc.scalar.activation(
                    out=sig, in_=lg_sb, func=mybir.ActivationFunctionType.Sigmoid
                )
                bc_ps = ps_rt.tile([128, E], FP32, tag="pr", name=f"bc_{b}")
                nc.tensor.matmul(bc_ps, ones_row, sig, start=True, stop=True)
                rw_sb = small.tile([128, E], FP32, tag="rws", name=f"rws_{b}")
                nc.scalar.copy(out=rw_sb, in_=bc_ps)

                # combined per-sample conv weights
                wb = wpool.tile([128, KH * KW, COUT], FP32, tag="wb", name=f"wb_{b}")
                wb_f = wb.rearrange("p a b -> p (a b)")
                nc.vector.tensor_scalar_mul(
                    out=wb_f.bitcast(F32R),
                    in0=e_r[:, 0, :, :].rearrange("p a b -> p (a b)"),
                    scalar1=rw_sb[:, 0:1],
                )
                for e in range(1, E):
                    nc.vector.scalar_tensor_tensor(
                        out=wb_f.bitcast(F32R),
                        in0=e_r[:, e, :, :].rearrange("p a b -> p (a b)"),
                        scalar=rw_sb[:, e : e + 1],
                        in1=wb_f,
                        op0=mybir.AluOpType.mult,
                        op1=mybir.AluOpType.add,
                    )

                # conv: 2 co_tiles x 8 row-blocks, 9-tap PSUM accumulation
                for ct in range(2):
                    st = stage.tile(
                        [128, NBLK, NN], FP32, tag="st", name=f"st_{b}_{ct}"
                    )
                    for g0, ng in ((0, 3), (3, 3), (6, 2)):
                        ps = ps_conv.tile(
                            [128, 3, 512], FP32, tag="pc", name=f"ps_{b}_{ct}_{g0}"
                        )
                        for kk in range(KH * KW):
                            dy, dx = divmod(kk, 3)
                            lhsT = wb[:, kk, ct * 128 : (ct + 1) * 128].bitcast(F32R)
                            for j in range(ng):
                                i = g0 + j
                                sl = slab0 if i < 4 else slab1
                                rr = 14 * (i % 4) + dy
                                rhs = sl[
                                    :, rr : rr + 13 : 2, dx : dx + 111 : 2
                                ].bitcast(F32R)
                                nc.tensor.matmul(
                                    ps[:, j, 0:NN],
                                    lhsT,
                                    rhs,
                                    start=(kk == 0),
                                    stop=(kk == KH * KW - 1),
                                )
                        nc.scalar.copy(
                            out=st[:, g0 : g0 + ng, :], in_=ps[:, 0:ng, 0:NN]
                        )
                    yv = y[b, ct * 128 : (ct + 1) * 128, :, :].rearrange(
                        "p a b -> p (a b)"
                    )
                    stv = st.rearrange("p a b -> p (a b)")
                    nc.sync.dma_start(out=yv[:, 0 : 4 * NN], in_=stv[:, 0 : 4 * NN])
                    nc.sync.dma_start(
                        out=yv[:, 4 * NN : 8 * NN], in_=stv[:, 4 * NN : 8 * NN]
                    )

    nc.finalize()
    _NC_CACHE = nc
    return nc


def make_in_maps(x, routing_w, routing_b, expert_weight):
    x = np.ascontiguousarray(np.asarray(x, dtype=np.float32))
    routing_w = np.ascontiguousarray(np.asarray(routing_w, dtype=np.float32))
    routing_b = np.ascontiguousarray(np.asarray(routing_b, dtype=np.float32))
    expert_weight = np.ascontiguousarray(np.asarray(expert_weight, dtype=np.float32))
    return [
        {
            "x": np.ascontiguousarray(x[c * B_SH : (c + 1) * B_SH]),
            "routing_w": routing_w,
            "routing_b": routing_b,
            "expert_weight": expert_weight,
        }
        for c in range(N_CORES)
    ]


def kernel(x, routing_w, routing_b, expert_weight):
    nc = build_nc()
    in_maps = make_in_maps(x, routing_w, routing_b, expert_weight)
    res = run_bass_kernel_spmd(nc, in_maps, core_ids=list(range(N_CORES)))
    return np.concatenate([res.results[c]["y"] for c in range(N_CORES)], axis=0)


# revision 18
# speedup vs baseline: 1.4447x; 1.4447x over previous
"""Trainium2 Bass/Tile kernel: MoE-routed per-sample dynamic 3x3 conv (stride 2).

Reference computation:
    pooled  = mean(x, HW)                                        (B, Cin)
    rw      = sigmoid(pooled @ routing_w.T + routing_b)          (B, E)
    kernels = einsum('be,eoihw->boihw', rw, expert_weight)       (B,Cout,Cin,3,3)
    y[b]    = conv2d(x[b], kernels[b], stride 2, pad 1)          (B,Cout,56,56)

Sharding: data-parallel over batch across 8 NeuronCores (4 samples each);
routing/expert weights replicated.  No collectives.

Per-core plan:
  - expert_weight is loaded in natural [co, ci*9] layout and PE-transposed
    once into [ci, e, tap, co] (lhsT layout for the conv matmuls).
  - per sample: x lives in SBUF as two half-sample "slabs" [128ci, 57, 128].
    Each slab row r holds x row r at columns 16..127; the load reads
    overlapping 128-element (512 B) runs from DRAM (stride 112) so every DMA
    descriptor is exactly 512 B -> full DMA bandwidth.  Column 15 is memset
    to zero and serves as the conv's left pad (iw = -1); columns 0..14 are
    don't-care bytes from the preceding DRAM row.
  - routing: pooled-col x routing_w^T matmul -> [1,E] logits; sigmoid on ACT;
    a K=1 ones-matmul broadcasts the 4 gate scalars to all 128 partitions.
  - combined per-sample conv weights: W_b = sum_e rw[b,e] * E_r[e] on GpSimd.
  - conv: out[co, oh, ow] accumulated in PSUM over the 9 taps; each matmul is
    lhsT=[ci,co_tile] (float32r), rhs = strided slab view [ci, 7 rows, 56 cols]
    (N=392 >= 256 keeps float32r at the full 1 row/cycle PE rate).  The top
    pad row (ih = -1) is handled by skipping output row 0 in block 0's dy=0
    taps; tap (1,1) runs first so start=True initializes the whole PSUM block.
"""

import numpy as np

try:
    import concourse.bass as bass
except ImportError:  # toolchain not on sys.path in a fresh interpreter
    import sys

    for _p in ("/opt/trn_rl_repo", "/root/.axon_site/_ro/trn_rl_repo"):
        if _p not in sys.path:
            sys.path.insert(0, _p)
    import concourse.bass as bass

import concourse.mybir as mybir
from concourse.bacc import Bacc
from concourse.bass_utils import run_bass_kernel_spmd
from concourse.masks import make_identity
from concourse.tile import TileContext

FP32 = mybir.dt.float32
F32R = mybir.dt.float32r

N_CORES = 8
B_FULL = 32
B_SH = B_FULL // N_CORES  # 4 samples per core
CIN = 128
H = W = 112
COUT = 256
E = 4
KH = KW = 3
OH = OW = 56
HWSZ = H * W  # 12544
R = 7  # output rows per PSUM block
NBLK = 8  # blocks per (sample, co_tile)
NN = R * OW  # 392 moving dim per matmul
S_ROWS = 57  # slab rows (slab0 uses 56: x rows 0..55; slab1: x rows 55..111)
S_COLS = 128  # row pitch: 15 dead + 1 zero-pad col + 112 data
PAD_C = 15  # zero pad column (conv iw = -1)
DATA_C = 16  # x column 0 lives here

# Tap order: (1,1) first — it covers every output element of every block, so
# start=True initializes the whole PSUM region before the dy=0 taps (which
# skip output row 0 in block 0) accumulate.
TAPS = [(1, 1), (1, 0), (1, 2), (0, 1), (0, 0), (0, 2), (2, 1), (2, 0), (2, 2)]

_NC_CACHE = None


def build_nc():
    global _NC_CACHE
    if _NC_CACHE is not None:
        return _NC_CACHE

    # Bacc (not raw Bass): its finalize() runs the legality passes this walrus
    # build needs — move_matmul_waits_to_ldweights + generate_event_semaphores
    # (max 1 sync wait per instruction) + register allocation.
    nc = Bacc(trn_type="TRN2")
    x = nc.dram_tensor("x", [B_SH, CIN, H, W], FP32, kind="ExternalInput")
    rw_h = nc.dram_tensor("routing_w", [E, CIN], FP32, kind="ExternalInput")
    rb_h = nc.dram_tensor("routing_b", [E], FP32, kind="ExternalInput")
    ew_h = nc.dram_tensor(
        "expert_weight", [E, COUT, CIN, KH, KW], FP32, kind="ExternalInput"
    )
    y = nc.dram_tensor("y", [B_SH, COUT, OH, OW], FP32, kind="ExternalOutput")

    with TileContext(nc) as tc:
        with (
            tc.tile_pool(name="const", bufs=1) as const,
            tc.tile_pool(name="slabs", bufs=4) as slabs,
            tc.tile_pool(name="wpool", bufs=2) as wpool,
            tc.tile_pool(name="stage", bufs=3) as stage,
            tc.tile_pool(name="small", bufs=2) as small,
            tc.tile_pool(name="ps_conv", bufs=2, space="PSUM") as ps_conv,
            tc.tile_pool(name="ps_rt", bufs=2, space="PSUM") as ps_rt,
        ):
            # ---------------- one-time prep ----------------
            identity = const.tile([128, 128], FP32)
            make_identity(nc, identity)
            ones_row = const.tile([1, 128], FP32)
            nc.vector.memset(ones_row, 1.0)
            bias_row = const.tile([1, E], FP32)
            nc.sync.dma_start(out=bias_row, in_=rb_h[:].unsqueeze(0))
            rw_nat = const.tile([E, CIN], FP32)
            nc.sync.dma_start(out=rw_nat, in_=rw_h[:, :])
            rwT_ps = ps_rt.tile([128, E], FP32, tag="pr")
            nc.tensor.transpose(rwT_ps, rw_nat, identity[0:E, 0:E])
            rwT = const.tile([128, E], FP32)
            nc.scalar.copy(out=rwT, in_=rwT_ps)

            # expert weights, transposed to lhsT layout [ci, e, tap, co]
            # (emitted ct-major so co_tile 0 becomes available first)
            e_r = const.tile([128, E, KH * KW, COUT], FP32)

            def emit_expert_prep():
                for ct in range(2):
                    for e in range(E):
                        chunk = stage.tile(
                            [128, CIN * KH * KW], FP32, tag="st", name=f"ew_{e}_{ct}"
                        )
                        nc.sync.dma_start(
                            out=chunk,
                            in_=ew_h[
                                e, ct * 128 : (ct + 1) * 128, :, :, :
                            ].rearrange("p a b c -> p (a b c)"),
                        )
                        ch_v = chunk.rearrange("p (ci k) -> p k ci", k=KH * KW)
                        for kk in range(KH * KW):
                            tp = ps_rt.tile(
                                [128, 128], FP32, tag="pr", name=f"tp_{e}_{ct}_{kk}"
                            )
                            nc.tensor.transpose(tp, ch_v[:, kk, :], identity)
                            dst = e_r[:, e, kk, ct * 128 : (ct + 1) * 128]
                            if kk % 2 == 0:
                                nc.vector.tensor_copy(out=dst, in_=tp)
                            else:
                                nc.scalar.copy(out=dst, in_=tp)

            # ---------------- per-sample pipeline ----------------
            # Emission is software-pipelined: sample b+1's loads + routing +
            # weight combine are emitted (= get scheduler priority) before
            # sample b's conv, so they execute under the previous conv.
            state = {}

            def emit_front(b):
                slab0 = slabs.tile(
                    [128, S_ROWS, S_COLS], FP32, tag="slab", name=f"slab0_{b}"
                )
                slab1 = slabs.tile(
                    [128, S_ROWS, S_COLS], FP32, tag="slab", name=f"slab1_{b}"
                )
                # Overlapped loads: slab row r cols 0..127 <- x flat
                # [112*r - 16, 112*r + 112): 512B descriptors at full DMA
                # rate.  slab0 rows 0..55 <- x rows 0..55; slab1 <- 55..111.
                # Each chunk is reduced for the global-avg-pool as it lands.
                xb = x[b, :, :, :].rearrange("p h w -> p (h w)")
                pooled = small.tile(
                    [128, 9], FP32, tag="pooled", name=f"pooled_{b}"
                )
                for c0 in range(4):
                    r0 = 14 * c0
                    if b == 0 and c0 == 0:
                        # x row 0 starts at the tensor base: no room for the
                        # 16-element lookback, fall back to the plain load.
                        nc.sync.dma_start(
                            out=slab0[:, 0:14, DATA_C:S_COLS].bitcast(F32R),
                            in_=x[b, :, 0:14, :].bitcast(F32R),
                        )
                    else:
                        src = bass.AP(
                            tensor=xb.tensor,
                            offset=xb.offset + 112 * r0 - DATA_C,
                            ap=[[HWSZ, 128], [112, 14], [1, S_COLS]],
                        )
                        nc.sync.dma_start(
                            out=slab0[:, r0 : r0 + 14, :].bitcast(F32R),
                            in_=src.bitcast(F32R),
                        )
                    nc.vector.tensor_reduce(
                        out=pooled[:, c0 : c0 + 1],
                        in_=slab0[:, r0 : r0 + 14, DATA_C:S_COLS],
                        axis=mybir.AxisListType.XY,
                        op=mybir.AluOpType.add,
                    )
                for c0 in range(4):
                    r0 = 14 * c0
                    nr = 14 if c0 < 3 else 15
                    src = bass.AP(
                        tensor=xb.tensor,
                        offset=xb.offset + 112 * (55 + r0) - DATA_C,
                        ap=[[HWSZ, 128], [112, nr], [1, S_COLS]],
                    )
                    nc.sync.dma_start(
                        out=slab1[:, r0 : r0 + nr, :].bitcast(F32R),
                        in_=src.bitcast(F32R),
                    )
                    # slab1 row 0 duplicates x row 55 -> skip it in the pool
                    rr0 = 1 if c0 == 0 else r0
                    nc.vector.tensor_reduce(
                        out=pooled[:, 4 + c0 : 5 + c0],
                        in_=slab1[:, rr0 : r0 + nr, DATA_C:S_COLS],
                        axis=mybir.AxisListType.XY,
                        op=mybir.AluOpType.add,
                    )
                # zero left-pad column (after the loads, which overwrite it)
                nc.gpsimd.memset(slab0[:, 0:56, PAD_C : PAD_C + 1], 0.0)
                nc.gpsimd.memset(slab1[:, :, PAD_C : PAD_C + 1], 0.0)

                nc.vector.tensor_reduce(
                    out=pooled[:, 8:9],
                    in_=pooled[:, 0:8],
                    axis=mybir.AxisListType.X,
                    op=mybir.AluOpType.add,
                )

                # routing gates
                lg_ps = ps_rt.tile([1, E], FP32, tag="pr", name=f"lg_{b}")
                nc.tensor.matmul(lg_ps, pooled[:, 8:9], rwT, start=True, stop=True)
                lg_sb = small.tile([1, E], FP32, tag="lg", name=f"lgs_{b}")
                nc.vector.scalar_tensor_tensor(
                    out=lg_sb,
                    in0=lg_ps,
                    scalar=1.0 / HWSZ,
                    in1=bias_row,
                    op0=mybir.AluOpType.mult,
                    op1=mybir.AluOpType.add,
                )
                sig = small.tile([1, E], FP32, tag="sig", name=f"sig_{b}")
                nc.scalar.activation(
                    out=sig, in_=lg_sb, func=mybir.ActivationFunctionType.Sigmoid
                )
                bc_ps = ps_rt.tile([128, E], FP32, tag="pr", name=f"bc_{b}")
                nc.tensor.matmul(bc_ps, ones_row, sig, start=True, stop=True)
                rw_sb = small.tile([128, E], FP32, tag="rws", name=f"rws_{b}")
                nc.scalar.copy(out=rw_sb, in_=bc_ps)

                # combined per-sample conv weights, in (tap-row, co_tile)
                # chunks ordered to match conv consumption, so the conv can
                # start as soon as the first chunk lands.
                wb = wpool.tile([128, KH * KW, COUT], FP32, tag="wb", name=f"wb_{b}")
                for ct in range(2):
                    for d in (1, 0, 2):
                        dstf = wb[:, 3 * d : 3 * d + 3, ct * 128 : (ct + 1) * 128]
                        srcs = [
                            e_r[:, e, 3 * d : 3 * d + 3, ct * 128 : (ct + 1) * 128]
                            for e in range(E)
                        ]
                        nc.vector.tensor_scalar_mul(
                            out=dstf.bitcast(F32R),
                            in0=srcs[0],
                            scalar1=rw_sb[:, 0:1],
                        )
                        for e in range(1, E):
                            nc.vector.scalar_tensor_tensor(
                                out=dstf.bitcast(F32R),
                                in0=srcs[e],
                                scalar=rw_sb[:, e : e + 1],
                                in1=dstf,
                                op0=mybir.AluOpType.mult,
                                op1=mybir.AluOpType.add,
                            )
                state[b] = (slab0, slab1, wb)

            def emit_conv(b):
                slab0, slab1, wb = state.pop(b)
                for ct in range(2):
                    for g0, ng in ((0, 3), (3, 3), (6, 2)):
                        ps = ps_conv.tile(
                            [128, 3, 512], FP32, tag="pc", name=f"ps_{b}_{ct}_{g0}"
                        )
                        for ti, (dy, dx) in enumerate(TAPS):
                            lhsT = wb[
                                :, dy * 3 + dx, ct * 128 : (ct + 1) * 128
                            ].bitcast(F32R)
                            for j in range(ng):
                                i = g0 + j
                                sl = slab0 if i < 4 else slab1
                                # slab row for output row r of block i:
                                #   14*(i%4) + 2r + dy (- 1 on slab0)
                                # block 0's dy=0 taps skip output row 0 (its
                                # input row is the all-zero top pad).
                                r_lo = 1 if (i == 0 and dy == 0) else 0
                                nr = R - r_lo
                                sr = (
                                    14 * (i % 4)
                                    + 2 * r_lo
                                    + dy
                                    - (1 if i < 4 else 0)
                                )
                                sc = PAD_C + dx
                                rhs = sl[
                                    :,
                                    sr : sr + 2 * nr - 1 : 2,
                                    sc : sc + 111 : 2,
                                ].bitcast(F32R)
                                nc.tensor.matmul(
                                    ps[:, j, r_lo * OW : NN],
                                    lhsT,
                                    rhs,
                                    start=(ti == 0),
                                    stop=(ti == KH * KW - 1),
                                )
                        # evict this group's blocks and write them out
                        st = stage.tile(
                            [128, 3, NN], FP32, tag="st", name=f"st_{b}_{ct}_{g0}"
                        )
                        nc.scalar.copy(out=st[:, 0:ng, :], in_=ps[:, 0:ng, 0:NN])
                        yv = y[b, ct * 128 : (ct + 1) * 128, :, :].rearrange(
                            "p a c -> p (a c)"
                        )
                        nc.sync.dma_start(
                            out=yv[:, g0 * NN : (g0 + ng) * NN],
                            in_=st[:, 0:ng, :].rearrange("p a c -> p (a c)"),
                        )

            emit_expert_prep()
            emit_front(0)
            for b in range(B_SH):
                if b + 1 < B_SH:
                    emit_front(b + 1)
                emit_conv(b)

    nc.finalize()
    _NC_CACHE = nc
    return nc


def make_in_maps(x, routing_w, routing_b, expert_weight):
    x = np.ascontiguousarray(np.asarray(x, dtype=np.float32))
    routing_w = np.ascontiguousarray(np.asarray(routing_w, dtype=np.float32))
    routing_b = np.ascontiguousarray(np.asarray(routing_b, dtype=np.float32))
    expert_weight = np.ascontiguousarray(np.asarray(expert_weight, dtype=np.float32))
    return [
        {
            "x": np.ascontiguousarray(x[c * B_SH : (c + 1) * B_SH]),
            "routing_w": routing_w,
            "routing_b": routing_b,
            "expert_weight": expert_weight,
        }
        for c in range(N_CORES)
    ]


def kernel(x, routing_w, routing_b, expert_weight):
    nc = build_nc()
    in_maps = make_in_maps(x, routing_w, routing_b, expert_weight)
    res = run_bass_kernel_spmd(nc, in_maps, core_ids=list(range(N_CORES)))
    return np.concatenate([res.results[c]["y"] for c in range(N_CORES)], axis=0)


# revision 37
# speedup vs baseline: 5.7462x; 3.9774x over previous
"""Trainium2 Bass/Tile kernel: MoE-routed per-sample dynamic 3x3 conv (stride 2).

Reference computation:
    pooled  = mean(x, HW)                                        (B, Cin)
    rw      = sigmoid(pooled @ routing_w.T + routing_b)          (B, E)
    kernels = einsum('be,eoihw->boihw', rw, expert_weight)       (B,Cout,Cin,3,3)
    y[b]    = conv2d(x[b], kernels[b], stride 2, pad 1)          (B,Cout,56,56)

Sharding: data-parallel over batch across 8 NeuronCores (4 samples each);
routing/expert weights replicated (host pre-transposes them into the conv
lhsT layout [ci, e, tap, co]).  No collectives.

Per-core plan (software-pipelined across the 4 samples):
  - x[b] lives in SBUF as two half-sample slabs [128ci, 59, 112]: two zero
    dummy rows + fully contiguous rows (multi-KB DMA descriptors at full HBM
    rate).  Per-chunk DVE reduces compute the global-avg-pool as chunks land.
  - routing: pooled-col x routing_w^T matmul -> [1,E] logits; sigmoid on ACT;
    a K=1 ones-matmul broadcasts the 4 gate scalars to all 128 partitions.
  - combined per-sample conv weights W_b = sum_e rw[b,e] * E_r[e] on DVE, in
    (tap-row, co_tile) chunks ordered so the conv can start on the first one.
  - conv: out[co, oh, ow] accumulated in PSUM over the 9 taps; each matmul is
    lhsT=[ci,co_tile] (float32r, 1 row/cycle at N=392), rhs = strided slab
    view [ci, 7 rows, 56 cols].  The top pad row (ih=-1) reads the zero dummy
    row; the left pad (iw=-1) reads the previous row's column 111, and a
    small correction matmul computes exactly that garbage term so it can be
    subtracted from output column 0 during eviction.
"""

import numpy as np

try:
    import concourse.bass as bass
except ImportError:  # toolchain not on sys.path in a fresh interpreter
    import sys

    for _p in ("/opt/trn_rl_repo", "/root/.axon_site/_ro/trn_rl_repo"):
        if _p not in sys.path:
            sys.path.insert(0, _p)
    import concourse.bass as bass

import concourse.mybir as mybir
from concourse.bacc import Bacc
from concourse.bass_utils import run_bass_kernel_spmd
from concourse.masks import make_identity
from concourse.tile import TileContext

FP32 = mybir.dt.float32
F32R = mybir.dt.float32r

N_CORES = 8
B_FULL = 32
B_SH = B_FULL // N_CORES  # 4 samples per core
CIN = 128
H = W = 112
COUT = 256
E = 4
KH = KW = 3
OH = OW = 56
HWSZ = H * W  # 12544
R = 7  # output rows per PSUM block
NBLK = 8  # blocks per (sample, co_tile)
NN = R * OW  # 392 moving dim per matmul
S_ROWS = 59  # 2 zero dummy rows + up to 57 data rows
S_COLS = 112  # fully contiguous rows (multi-KB DMA descriptors)
DROW = 2  # x data starts at this slab row

# Tap order matches the weight-combine chunk order (dy=1 first).
TAPS = [(1, 1), (1, 0), (1, 2), (0, 1), (0, 0), (0, 2), (2, 1), (2, 0), (2, 2)]

_NC_CACHE = {}


def build_nc(rep=1):
    """Build the per-core module.  rep > 1 repeats the whole pipeline (same
    inputs/outputs) — used only for benchmarking slope measurements."""
    if rep in _NC_CACHE:
        return _NC_CACHE[rep]

    # Bacc (not raw Bass): its finalize() runs the legality passes this walrus
    # build needs — move_matmul_waits_to_ldweights + generate_event_semaphores
    # (max 1 sync wait per instruction) + register allocation.
    nc = Bacc(trn_type="TRN2")
    x = nc.dram_tensor("x", [B_SH, CIN, H, W], FP32, kind="ExternalInput")
    # weights arrive pre-transposed from the host (see make_in_maps):
    #   ew_t: [ci, e, tap, co]  (conv lhsT layout)   rw_t: [ci, e]
    rwt_h = nc.dram_tensor("routing_wt", [CIN, E], FP32, kind="ExternalInput")
    rb_h = nc.dram_tensor("routing_b", [E], FP32, kind="ExternalInput")
    ewt_h = nc.dram_tensor(
        "expert_weight_t", [CIN, E, KH * KW, COUT], FP32, kind="ExternalInput"
    )
    y = nc.dram_tensor("y", [B_SH, COUT, OH, OW], FP32, kind="ExternalOutput")

    with TileContext(nc) as tc:
        with (
            tc.tile_pool(name="const", bufs=1) as const,
            tc.tile_pool(name="slabs", bufs=4) as slabs,
            tc.tile_pool(name="wpool", bufs=2) as wpool,
            tc.tile_pool(name="stage", bufs=3) as stage,
            tc.tile_pool(name="small", bufs=2) as small,
            tc.tile_pool(name="ps_conv", bufs=2, space="PSUM") as ps_conv,
            tc.tile_pool(name="ps_rt", bufs=2, space="PSUM") as ps_rt,
        ):
            # ---------------- one-time prep ----------------
            ones_row = const.tile([1, 128], FP32)
            nc.vector.memset(ones_row, 1.0)
            bias_row = const.tile([1, E], FP32)
            nc.sync.dma_start(out=bias_row, in_=rb_h[:].unsqueeze(0))
            rwT = const.tile([128, E], FP32)
            nc.sync.dma_start(out=rwT, in_=rwt_h[:, :])

            # expert weights in lhsT layout [ci, e, tap, co], loaded directly
            e_r = const.tile([128, E, KH * KW, COUT], FP32)

            def emit_expert_prep():
                # tap-group major, d=1 first: the conv consumes d=1 taps first
                for d in (1, 0, 2):
                    nc.sync.dma_start(
                        out=e_r[:, :, 3 * d : 3 * d + 3, :],
                        in_=ewt_h[:, :, 3 * d : 3 * d + 3, :],
                    )

            # ---------------- per-sample pipeline ----------------
            # Emission is software-pipelined: sample b+1's loads + routing +
            # weight combine are emitted (= get scheduler priority) before
            # sample b's conv, so they execute under the previous conv.
            state = {}
            gstate = {}

            def emit_loads(b):
                slab0 = slabs.tile(
                    [128, S_ROWS, S_COLS], FP32, tag="slab", name=f"slab0_{b}"
                )
                slab1 = slabs.tile(
                    [128, S_ROWS, S_COLS], FP32, tag="slab", name=f"slab1_{b}"
                )
                # Fully contiguous loads (descriptors of 14 rows = 6.3 KB):
                #   slab0 rows 2..57 <- x rows 0..55; slab1 rows 2..58 <- 55..111
                # Rows 0..1 are memset to zero: row 1 is the conv's top pad
                # (ih = -1) and row DROW-1's column 111 doubles as the left pad
                # (iw = -1) for the first data row.  For the remaining rows the
                # dx=0 taps read the previous row's column 111 (garbage); a
                # per-co-tile correction matmul subtracts exactly that term
                # from output column 0 later.
                nc.gpsimd.memset(slab0[:, 0:DROW, :], 0.0)
                nc.gpsimd.memset(slab1[:, 0:DROW, :], 0.0)
                pooled = small.tile(
                    [128, 9], FP32, tag="pooled", name=f"pooled_{b}"
                )
                for c0 in range(4):
                    r0 = 14 * c0
                    nc.sync.dma_start(
                        out=slab0[:, DROW + r0 : DROW + r0 + 14, :].bitcast(F32R),
                        in_=x[b % B_SH, :, r0 : r0 + 14, :].bitcast(F32R),
                    )
                    nc.vector.tensor_reduce(
                        out=pooled[:, c0 : c0 + 1],
                        in_=slab0[:, DROW + r0 : DROW + r0 + 14, :],
                        axis=mybir.AxisListType.XY,
                        op=mybir.AluOpType.add,
                    )
                for c0 in range(4):
                    r0 = 14 * c0
                    nr = 14 if c0 < 3 else 15
                    nc.sync.dma_start(
                        out=slab1[:, DROW + r0 : DROW + r0 + nr, :].bitcast(F32R),
                        in_=x[b % B_SH, :, 55 + r0 : 55 + r0 + nr, :].bitcast(F32R),
                    )
                    # slab1 row DROW duplicates x row 55 -> skip it in the pool
                    rr0 = DROW + 1 if c0 == 0 else DROW + r0
                    nc.vector.tensor_reduce(
                        out=pooled[:, 4 + c0 : 5 + c0],
                        in_=slab1[:, rr0 : DROW + r0 + nr, :],
                        axis=mybir.AxisListType.XY,
                        op=mybir.AluOpType.add,
                    )
                gstate[b] = pooled
                state[b] = (slab0, slab1, None)

            def emit_gates(b):
                pooled = gstate.pop(b)
                slab0, slab1, _ = state[b]
                nc.vector.tensor_reduce(
                    out=pooled[:, 8:9],
                    in_=pooled[:, 0:8],
                    axis=mybir.AxisListType.X,
                    op=mybir.AluOpType.add,
                )

                # routing gates
                lg_ps = ps_rt.tile([1, E], FP32, tag="pr", name=f"lg_{b}")
                nc.tensor.matmul(lg_ps, pooled[:, 8:9], rwT, start=True, stop=True)
                lg_sb = small.tile([1, E], FP32, tag="lg", name=f"lgs_{b}")
                nc.vector.scalar_tensor_tensor(
                    out=lg_sb,
                    in0=lg_ps,
                    scalar=1.0 / HWSZ,
                    in1=bias_row,
                    op0=mybir.AluOpType.mult,
                    op1=mybir.AluOpType.add,
                )
                sig = small.tile([1, E], FP32, tag="sig", name=f"sig_{b}")
                nc.scalar.activation(
                    out=sig, in_=lg_sb, func=mybir.ActivationFunctionType.Sigmoid
                )
                bc_ps = ps_rt.tile([128, E], FP32, tag="pr", name=f"bc_{b}")
                nc.tensor.matmul(bc_ps, ones_row, sig, start=True, stop=True)
                rw_sb = small.tile([128, E], FP32, tag="rws", name=f"rws_{b}")
                nc.scalar.copy(out=rw_sb, in_=bc_ps)

                # combined per-sample conv weights, in (tap-row, co_tile)
                # chunks ordered to match conv consumption, so the conv can
                # start as soon as the first chunk lands.
                wb = wpool.tile([128, KH * KW, COUT], FP32, tag="wb", name=f"wb_{b}")
                for ct in range(2):
                    for d in (1, 0, 2):
                        dstf = wb[:, 3 * d : 3 * d + 3, ct * 128 : (ct + 1) * 128]
                        srcs = [
                            e_r[:, e, 3 * d : 3 * d + 3, ct * 128 : (ct + 1) * 128]
                            for e in range(E)
                        ]
                        nc.vector.tensor_scalar_mul(
                            out=dstf.bitcast(F32R),
                            in0=srcs[0],
                            scalar1=rw_sb[:, 0:1],
                        )
                        for e in range(1, E):
                            nc.vector.scalar_tensor_tensor(
                                out=dstf.bitcast(F32R),
                                in0=srcs[e],
                                scalar=rw_sb[:, e : e + 1],
                                in1=dstf,
                                op0=mybir.AluOpType.mult,
                                op1=mybir.AluOpType.add,
                            )
                state[b] = (slab0, slab1, wb)

            cstate = {}

            def emit_corrections(b):
                slab0, slab1, wb = state[b]
                allc = []
                for ct in range(2):
                    # dx=0 garbage corrections: C[co, oh] = sum_dy W(dy,0)^T .
                    # slab[prev-row col 111], one 28-wide run per half-sample.
                    c_sb = []
                    for run, sl in enumerate((slab0, slab1)):
                        c_ps = ps_rt.tile(
                            [128, 28], FP32, tag="pr", name=f"cps_{b}_{ct}_{run}"
                        )
                        fv = sl[:]
                        for di, dy in enumerate((0, 1, 2)):
                            lhsT = wb[
                                :, dy * 3, ct * 128 : (ct + 1) * 128
                            ].bitcast(F32R)
                            # garbage row for output row oh: 2*oh + dy (+1 on
                            # slab1) + (DROW - 1), column 111
                            roff = dy + (1 if run else 0)
                            rhs = bass.AP(
                                tensor=fv.tensor,
                                offset=fv.offset + roff * S_COLS + 111,
                                ap=[[fv.ap[0][0], 128], [2 * S_COLS, 28]],
                            ).bitcast(F32R)
                            nc.tensor.matmul(
                                c_ps[:, 0:28],
                                lhsT,
                                rhs,
                                start=(di == 0),
                                stop=(di == 2),
                            )
                        cs = small.tile(
                            [128, 28],
                            FP32,
                            tag="csb",
                            bufs=4,
                            name=f"cs_{b}_{ct}_{run}",
                        )
                        nc.scalar.copy(out=cs, in_=c_ps)
                        c_sb.append(cs)
                    allc.append(c_sb)
                cstate[b] = allc

            def emit_conv(b):
                slab0, slab1, wb = state.pop(b)
                allc = cstate.pop(b)
                for ct in range(2):
                    c_sb = allc[ct]
                    for g0, ng in ((0, 3), (3, 3), (6, 2)):
                        ps = ps_conv.tile(
                            [128, 3, 512], FP32, tag="pc", name=f"ps_{b}_{ct}_{g0}"
                        )
                        for ti, (dy, dx) in enumerate(TAPS):
                            lhsT = wb[
                                :, dy * 3 + dx, ct * 128 : (ct + 1) * 128
                            ].bitcast(F32R)
                            for j in range(ng):
                                i = g0 + j
                                sl = slab0 if i < 4 else slab1
                                fv = sl[:]
                                # slab row for output row r of block i:
                                #   14*(i%4) + 2r + dy + DROW - 1 (slab0)
                                #   14*(i%4) + 2r + dy + DROW     (slab1)
                                sr = (
                                    14 * (i % 4)
                                    + dy
                                    + (DROW - 1 if i < 4 else DROW)
                                )
                                rhs = bass.AP(
                                    tensor=fv.tensor,
                                    offset=fv.offset + sr * S_COLS + dx - 1,
                                    ap=[
                                        [fv.ap[0][0], 128],
                                        [2 * S_COLS, R],
                                        [2, OW],
                                    ],
                                ).bitcast(F32R)
                                nc.tensor.matmul(
                                    ps[:, j, 0:NN],
                                    lhsT,
                                    rhs,
                                    start=(ti == 0),
                                    stop=(ti == KH * KW - 1),
                                )
                        # evict, subtract the dx=0 garbage from column 0 of
                        # each output row, then write out
                        st = stage.tile(
                            [128, 3, NN], FP32, tag="st", name=f"st_{b}_{ct}_{g0}"
                        )
                        nc.scalar.copy(out=st[:, 0:ng, :], in_=ps[:, 0:ng, 0:NN])
                        col0 = st[:, 0:ng, 0 : 6 * OW + 1 : OW]
                        if g0 == 0:
                            fix = [(col0, c_sb[0][:, 0:21], 3)]
                        elif g0 == 3:
                            fix = [
                                (st[:, 0:1, 0 : 6 * OW + 1 : OW], c_sb[0][:, 21:28], 1),
                                (st[:, 1:3, 0 : 6 * OW + 1 : OW], c_sb[1][:, 0:14], 2),
                            ]
                        else:
                            fix = [(st[:, 0:2, 0 : 6 * OW + 1 : OW], c_sb[1][:, 14:28], 2)]
                        for dst_v, c_v, nb_ in fix:
                            nc.vector.tensor_sub(
                                out=dst_v,
                                in0=dst_v,
                                in1=c_v.rearrange("p (a c) -> p a c", c=R),
                            )
                        yv = y[b % B_SH, ct * 128 : (ct + 1) * 128, :, :].rearrange(
                            "p a c -> p (a c)"
                        )
                        nc.sync.dma_start(
                            out=yv[:, g0 * NN : (g0 + ng) * NN],
                            in_=st[:, 0:ng, :].rearrange("p a c -> p (a c)"),
                        )

            nb = B_SH * rep
            emit_loads(0)
            emit_expert_prep()
            emit_gates(0)
            for b in range(nb):
                emit_corrections(b)
                if b + 1 < nb:
                    emit_loads(b + 1)
                    emit_gates(b + 1)
                emit_conv(b)

    nc.finalize()
    _NC_CACHE[rep] = nc
    return nc


def make_in_maps(x, routing_w, routing_b, expert_weight):
    x = np.ascontiguousarray(np.asarray(x, dtype=np.float32))
    routing_w = np.asarray(routing_w, dtype=np.float32)
    routing_b = np.ascontiguousarray(np.asarray(routing_b, dtype=np.float32))
    expert_weight = np.asarray(expert_weight, dtype=np.float32)
    # host-side weight re-layout (replicated across cores):
    #   expert_weight [e, co, ci, kh, kw] -> [ci, e, kh*kw, co]
    ew_t = np.ascontiguousarray(
        expert_weight.transpose(2, 0, 3, 4, 1).reshape(CIN, E, KH * KW, COUT)
    )
    rw_t = np.ascontiguousarray(routing_w.T)
    return [
        {
            "x": np.ascontiguousarray(x[c * B_SH : (c + 1) * B_SH]),
            "routing_wt": rw_t,
            "routing_b": routing_b,
            "expert_weight_t": ew_t,
        }
        for c in range(N_CORES)
    ]


def kernel(x, routing_w, routing_b, expert_weight):
    nc = build_nc()
    in_maps = make_in_maps(x, routing_w, routing_b, expert_weight)
    res = run_bass_kernel_spmd(nc, in_maps, core_ids=list(range(N_CORES)))
    return np.concatenate([res.results[c]["y"] for c in range(N_CORES)], axis=0)
